# revision 43
# baseline (speedup 1.0000x reference)
"""Trainium2 Bass kernel for nn_BiLSTM_CRF_18098992185950 (8 NeuronCores).

Math reformulation (validated against the jax reference):

  conv(2ch,k3,p1) + Linear(D->1) collapse into fixed 256-d projection vectors:
      dot(l, conv1ch(x, w)) = dot(g, x),  g[d] = w0*l[d+1] + w1*l[d] + w2*l[d-1]
  so per-candidate scores are dots with 3 fixed table-projection vectors
      b = E[id].g_e1 (emit, cand), u = E[id].g_t0 (trans prev),
      v = E[id].g_t1 (trans cur), plus a = obs_t.g_e0 (emit, obs, in L2)
  emit[t,k] = sigmoid(a_t + b_tk + ce);  trans = sigmoid(u + v + ct)

  Sigmoids are computed as tanh (sigma(x) = (1+tanh(x/2))/2) so the whole
  kernel uses one ACT table set (tanh+exp); the affine corrections fold into
  staged constants and the exp's free scale.

  The CRF forward DP in normal space is a matrix-product chain:
      Z = 1^T (prod_{t=0}^{1022} A_t) exp(emit_{1023}),
      A_t[j,k] = exp(sigmoid(u_t[j] + v_{t+1}[k] + ct) + emit_t[j] - log s)
  Products are associative -> 256 subchains of 4 leaves (1023 real + one
  identity pad); the host combines 256 64x64 matrices in f64.

Launch 1 streams the deduplicated embedding table (~48k unique rows of the
100k vocab, host pre-transposed to (128, 2ch, cols) bf16) and computes the
three projections per row directly on the PE (G stationary, table moving;
memory-bound).  The host gathers proj[candidate_ids] (pure indexing).
Launch 2 is T-parallel: leaf pair-blocks stacked into 128 partitions (leaf
t_top on parts 0:64, t_top+64 on 64:128), built by 16 N=512 matmuls against
a host-staged [u-broadcast ; v] operand, then 32 subchains x 4 rounds of
64x64 chain matmuls.  Leaf blocks are permuted so chain round i reads blocks
16i..16i+15.  Both launches warm the PE (HAM clock gate) during the input
DMA with throwaway matmuls."""

import numpy as np

T = 1024
K = 64
D = 256
V = 100000
NCORES = 8
NT = 128           # frames per core in L2
NSUB = 32          # subchains per core
LSUB = 4           # leaves per subchain (NSUB*LSUB == NT)
NPAIR = NSUB // 2  # stacked subchain pairs
NBLK = 64          # leaf pair-blocks per core (NT // 2)
L1_CHUNK = 2048    # table columns per streamed DMA chunk
L1_WARM = 70       # PE warm-up matmuls in L1 (tiny N=3, ~50ns each cold)

# blob byte offsets (per partition); qinit ships separately on the scalar
# queue so the blob (which gates the whole P2 pipeline) stays small
B_ID, B_BT, B_CV, B_ADD, B_OBS, B_GE, B_END = (
    0, 512, 768, 784, 1040, 1552, 1556)

_PROG = {}


def _gvec(w3, l):
    g = np.zeros_like(l)
    g += w3[1] * l
    g[:-1] += w3[0] * l[1:]
    g[1:] += w3[2] * l[:-1]
    return g


def _mods():
    import concourse.bacc as bacc
    import concourse.mybir as mybir
    from concourse import tile
    return bacc, mybir, tile


def _build_p1(vshc):
    key = ("p1", vshc)
    if key in _PROG:
        return _PROG[key]
    bacc, mybir, tile = _mods()
    f32 = mybir.dt.float32
    fp8 = mybir.dt.float8e4

    nc = bacc.Bacc("TRN2", target_bir_lowering=False, debug=False,
                   enable_asserts=False, num_devices=NCORES)
    # etab[p, ch, r] = E[uniq[shard r], ch*128 + p] * 16  (fp8; DoubleRow
    # pairs the middle Ko=2 dim on both operands)
    etab = nc.dram_tensor("etab", (128, 2, vshc), fp8, kind="ExternalInput").ap()
    gmat = nc.dram_tensor("gmat", (128, 2, 16), fp8, kind="ExternalInput").ap()
    projout = nc.dram_tensor("projout", (3, vshc), f32, kind="ExternalOutput").ap()

    chunks = []
    c0 = 0
    while c0 < vshc:
        w = min(L1_CHUNK, vshc - c0)
        chunks.append((c0, w))
        c0 += w

    with tile.TileContext(nc) as tc:
        with (
            tc.tile_pool(name="persist", bufs=1) as pp,
            tc.tile_pool(name="load", bufs=3) as lp,
            tc.tile_pool(name="out", bufs=3) as op,
            tc.tile_pool(name="ps", bufs=3, space="PSUM") as ps,
            tc.tile_pool(name="ps_w", bufs=1, space="PSUM") as ps_w,
        ):
            # table chunks stream on the sync HWDGE queue; the small gmat
            # goes via the scalar HWDGE queue so it lands first and the PE
            # can warm up (HAM clock gate) during the big DMAs.
            for ci, (c0, w) in enumerate(chunks):
                ld = lp.tile([128, 2, L1_CHUNK], fp8, tag="ld")
                nc.sync.dma_start(ld[:, :, :w], etab[:, :, c0 : c0 + w])
                if ci == 0:
                    g_sb = pp.tile([128, 2, 16], fp8, tag="gmat")
                    nc.scalar.dma_start(g_sb[:], gmat)
                    wps = ps_w.tile([16, 16], f32, tag="wps")
                    for _ in range(L1_WARM):
                        nc.tensor.matmul(out=wps[:], lhsT=g_sb[:, 0, :],
                                         rhs=g_sb[:, 0, :], start=True,
                                         stop=True)
                osb = op.tile([3, L1_CHUNK], f32, tag="osb")
                for s0 in range(0, w, 1024):
                    sw = min(1024, w - s0)
                    pj = ps.tile([16, 1024], f32, tag="pj")
                    for b0 in range(0, sw, 512):
                        bw = min(512, sw - b0)
                        nc.tensor.matmul(
                            out=pj[:, b0 : b0 + bw],
                            lhsT=g_sb[:],
                            rhs=ld[:, :, s0 + b0 : s0 + b0 + bw],
                            start=True, stop=True,
                            perf_mode=mybir.MatmulPerfMode.DoubleRow,
                        )
                    if (s0 // 1024) % 2 == 0:
                        nc.vector.tensor_copy(out=osb[:, s0 : s0 + sw],
                                              in_=pj[0:3, :sw])
                    else:
                        nc.scalar.copy(out=osb[:, s0 : s0 + sw], in_=pj[0:3, :sw])
                nc.sync.dma_start(out=projout[:, c0 : c0 + w], in_=osb[:, :w])
    nc.compile()
    _PROG[key] = nc
    return nc


def _build_p2():
    if "p2" in _PROG:
        return _PROG["p2"]
    bacc, mybir, tile = _mods()
    f32 = mybir.dt.float32
    bf16 = mybir.dt.bfloat16
    u8 = mybir.dt.uint8
    AF = mybir.ActivationFunctionType
    OP = mybir.AluOpType

    nc = bacc.Bacc("TRN2", target_bir_lowering=False, debug=False,
                   enable_asserts=False, num_devices=NCORES)
    blobin = nc.dram_tensor("blobin", (128, B_END), u8, kind="ExternalInput").ap()
    qinit = nc.dram_tensor("qinit", (128, NPAIR * K), bf16,
                           kind="ExternalInput").ap()
    # uv = [uvtop (65,4096) | uvbot (65,4096) | iones (65,64)]
    uvin = nc.dram_tensor("uvin", (65, 8256), bf16, kind="ExternalInput").ap()
    qout = nc.dram_tensor("qout", (128, NPAIR * K), f32, kind="ExternalOutput").ap()
    emitout = nc.dram_tensor("emitout", (NT, K), f32, kind="ExternalOutput").ap()

    with tile.TileContext(nc) as tc:
        with (
            tc.tile_pool(name="persist", bufs=1) as pp,
            tc.tile_pool(name="sig", bufs=2) as gp,
            tc.tile_pool(name="ps_misc", bufs=1, space="PSUM") as ps_misc,
            tc.tile_pool(name="ps_leaf", bufs=2, space="PSUM") as ps_leaf,
            tc.tile_pool(name="ps_q", bufs=2, space="PSUM") as ps_q,
        ):
            # all input DMAs on ONE queue: a single queue's packets drain in
            # order, so chunk it lands before chunk it+1 (two queues would
            # round-robin at packet granularity and finish together)
            blob = pp.tile([128, B_END], u8, tag="blob")
            nc.sync.dma_start(blob[:], blobin)
            qbig = pp.tile([128, NPAIR * K], bf16, tag="qbig")
            nc.scalar.dma_start(qbig[:], qinit)
            uv = pp.tile([65, 8256], bf16, tag="uv")
            nc.sync.dma_start(uv[:, 8192:8256], uvin[:, 8192:8256])
            for ck in range(4):
                nc.sync.dma_start(uv[:, ck * 2048 : (ck + 1) * 2048],
                                  uvin[:, ck * 2048 : (ck + 1) * 2048])

            id_sb = blob[:, B_ID:B_BT].bitcast(f32)             # (128, 128)
            bt = blob[:, B_BT:B_CV].bitcast(f32)                # (128, 64)
            cols = blob[:, B_CV:B_ADD].bitcast(f32)             # (128, 4)
            ct2_col, ce2_col = cols[:, 0:1], cols[:, 1:2]
            m2_col, mask_col = cols[:, 2:3], cols[:, 3:4]
            add_sb = blob[:, B_ADD:B_OBS].bitcast(f32)          # (128, 64)
            obs_sb = blob[:, B_OBS:B_GE].bitcast(bf16).rearrange(
                "p (c t) -> p c t", c=2)                        # (128, 2, 128)
            ge_sb = blob[:, B_GE:B_END].bitcast(bf16).rearrange(
                "p (c o) -> p c o", c=2)                        # (128, 2, 1)
            iones = uv[:, 8192:8256]                            # (65, 64)

            # emit path at high priority: it feeds every STT via emitc2, so
            # the scheduler must not queue it behind the leaf tanh passes
            with tc.high_priority():
                # a-column: a[t] = obs_t . g_e0; acol2 = (a + ce)/2
                acps = ps_misc.tile([128, 8], f32, tag="acps")
                for ch in range(2):
                    nc.tensor.matmul(out=acps[:, 0:1], lhsT=obs_sb[:, ch, :],
                                     rhs=ge_sb[:, ch, :],
                                     start=(ch == 0), stop=(ch == 1))
                acol2 = pp.tile([128, 1], f32, tag="acol2")
                nc.scalar.activation(acol2[:], acps[:, 0:1], AF.Identity,
                                     bias=ce2_col, scale=0.5)
                # short PE warm-up burst (HAM clock gate) while ACT runs the
                # emit path; keeps the leaf matmuls from starting fully cold
                for _ in range(10):
                    nc.tensor.matmul(out=acps[:, 1:8], lhsT=id_sb,
                                     rhs=id_sb[:, 0:7], start=True, stop=True)

                # emit2[t,j] = tanh((b + a + ce)/2) = 2*emit - 1
                emit2 = pp.tile([NT, K], f32, tag="emit2")
                nc.scalar.activation(emit2[:], bt, AF.Tanh, bias=acol2[:],
                                     scale=0.5)
                nc.scalar.dma_start(out=emitout, in_=emit2[:])
                # emitc2[p, i*NPAIR+g] = emit2[t_top + 64*(p>=64), p%64] via
                # PE matmuls against a column-permuted identity
                ecps = ps_misc.tile([128, NBLK], f32, tag="ecps")
                idp_t = id_sb[0:K, 0:K].rearrange("p (g i) -> p i g", g=NPAIR)
                idp_b = id_sb[K:128, K:128].rearrange("p (g i) -> p i g",
                                                     g=NPAIR)
                nc.tensor.matmul(out=ecps[0:K, :], lhsT=emit2[0:K, :],
                                 rhs=idp_t, start=True, stop=True)
                nc.tensor.matmul(out=ecps[K:128, :], lhsT=emit2[K:128, :],
                                 rhs=idp_b, start=True, stop=True,
                                 tile_position=(64, 64))
                emitc2 = pp.tile([128, NBLK], bf16, tag="emitc2")
                nc.vector.tensor_copy(out=emitc2[:], in_=ecps[:])

            # leaves: block beta=16i+g holds leaf t_top=4g+i (parts 0:64)
            # and leaf t_top+64 (parts 64:128); uvtop/uvbot staged by beta.
            # ACT issue order interleaves tanh/exp: t0 t1 e0 t2 e1 t3 e2 e3.
            stage2 = pp.tile([128, NBLK * K], bf16, tag="stage2")
            leafbuf = pp.tile([128, NBLK * K], bf16, tag="leafbuf")

            def emit_exp(j):
                # leaf = exp(stage2 / 2); exp_j covers blocks 16j..16j+16,
                # exactly chain round j's operands
                nc.scalar.activation(
                    leafbuf[:, j * 1024 : (j + 1) * 1024],
                    stage2[:, j * 1024 : (j + 1) * 1024], AF.Exp, scale=0.5)

            for it in range(4):
                pl = ps_leaf.tile([128, 1024], f32, tag="pl")
                for half in range(2):
                    c0 = it * 2048 + half * 512
                    nc.tensor.matmul(
                        out=pl[0:K, half * 512 : half * 512 + 512],
                        lhsT=iones, rhs=uv[:, c0 : c0 + 512],
                        start=True, stop=True)
                    nc.tensor.matmul(
                        out=pl[K:128, half * 512 : half * 512 + 512],
                        lhsT=iones, rhs=uv[:, c0 + 1024 : c0 + 1536],
                        start=True, stop=True, tile_position=(0, 64))
                sig = gp.tile([128, 1024], bf16, tag="sig")
                nc.scalar.activation(sig[:], pl[:], AF.Tanh,
                                     bias=ct2_col, scale=0.5)
                nc.vector.scalar_tensor_tensor(
                    out=stage2[:, it * 1024 : (it + 1) * 1024].rearrange(
                        "p (t k) -> p t k", k=K),
                    in0=sig[:].rearrange("p (t k) -> p t k", k=K),
                    scalar=m2_col,
                    in1=emitc2[:, it * 16 : (it + 1) * 16].unsqueeze(
                        2).to_broadcast((128, 16, K)),
                    op0=OP.add, op1=OP.add,
                )
                if it >= 1:
                    emit_exp(it - 1)
            emit_exp(3)

            # chain: pair g = subchains (g, g+16); round i uses block 16i+g
            for i in range(LSUB):
                if i == LSUB - 1:
                    # pad leaf (block 63, bottom half): leaf*mask + addend.
                    # Emitted here so it sits after rounds 0-2's evicts in
                    # the DVE FIFO (it waits on the last exp).
                    last = leafbuf[:, (NBLK - 1) * K : NBLK * K]
                    nc.vector.scalar_tensor_tensor(
                        out=last, in0=last, scalar=mask_col, in1=add_sb,
                        op0=OP.mult, op1=OP.add,
                    )
                for half in range(2):
                    pq = ps_q.tile([128, 512], f32, tag="pq")
                    for gg in range(8):
                        g = half * 8 + gg
                        bb = NPAIR * i + g
                        nc.tensor.matmul(
                            out=pq[0:K, gg * K : (gg + 1) * K],
                            lhsT=leafbuf[0:K, bb * K : (bb + 1) * K],
                            rhs=qbig[0:K, g * K : (g + 1) * K],
                            start=True, stop=True)
                        nc.tensor.matmul(
                            out=pq[K:128, gg * K : (gg + 1) * K],
                            lhsT=leafbuf[K:128, bb * K : (bb + 1) * K],
                            rhs=qbig[K:128, g * K : (g + 1) * K],
                            start=True, stop=True, tile_position=(64, 64))
                    if i < LSUB - 1:
                        nc.vector.tensor_copy(
                            out=qbig[:, half * 512 : (half + 1) * 512], in_=pq[:])
                    else:
                        qo = pp.tile([128, 512], f32, tag=f"qout_sb{half}")
                        nc.vector.tensor_copy(out=qo[:], in_=pq[:])
                        nc.sync.dma_start(
                            out=qout[:, half * 512 : (half + 1) * 512],
                            in_=qo[:])
    nc.compile()
    _PROG["p2"] = nc
    return nc


def _host_consts(inputs):
    E = np.ascontiguousarray(np.asarray(inputs["word_embeds"], dtype=np.float32))
    ids = np.asarray(inputs["candidate_ids"]).astype(np.int64)
    obs = np.ascontiguousarray(np.asarray(inputs["observed_feats"], dtype=np.float32))

    lw_e = np.asarray(inputs["emit_lin_w"], dtype=np.float64)[0]
    lw_t = np.asarray(inputs["trans_lin_w"], dtype=np.float64)[0]
    cw_e = np.asarray(inputs["emit_conv_w"], dtype=np.float64)
    cw_t = np.asarray(inputs["trans_conv_w"], dtype=np.float64)
    g_e0 = _gvec(cw_e[0, 0], lw_e)
    g_e1 = _gvec(cw_e[0, 1], lw_e)
    g_t0 = _gvec(cw_t[0, 0], lw_t)
    g_t1 = _gvec(cw_t[0, 1], lw_t)
    ce = float(np.asarray(inputs["emit_conv_b"], np.float64)[0] * lw_e.sum()
               + np.asarray(inputs["emit_lin_b"], np.float64)[0])
    ct = float(np.asarray(inputs["trans_conv_b"], np.float64)[0] * lw_t.sum()
               + np.asarray(inputs["trans_lin_b"], np.float64)[0])
    gmat = np.stack([g_e1, g_t0, g_t1], axis=1).astype(np.float32)  # (D, 3)

    samp = E[ids[:8].ravel()].astype(np.float64)
    sig = 1.0 / (1.0 + np.exp(-((samp @ g_t0).mean() + (samp @ g_t1).mean() + ct)))
    a8 = obs[:8].astype(np.float64) @ g_e0
    em = 1.0 / (1.0 + np.exp(-(a8.mean() + (samp @ g_e1).mean() + ce)))
    s = float(64.0 * np.exp(sig + em))
    return E, ids, obs, gmat, g_e0.astype(np.float32), ce, ct, s


def _run_launches(inputs, run_kw1=None, run_kw2=None):
    """Run both launches; returns (answer, res1, res2)."""
    import ml_dtypes
    from concourse.bass_utils import run_bass_kernel_spmd

    bf16 = ml_dtypes.bfloat16
    run_kw1 = run_kw1 or {}
    run_kw2 = run_kw2 or {}
    E, ids, obs, gmat, g_e0, ce, ct, s = _host_consts(inputs)

    # ---- dedup + launch 1: proj = E[uniq] @ G, sharded over unique rows ----
    ids_pad = np.zeros((T + 1, K), dtype=np.int64)
    ids_pad[:T] = ids
    uniq, inv = np.unique(ids_pad.ravel(), return_inverse=True)
    nu = len(uniq)
    nu_pad = -(-nu // (NCORES * 1024)) * (NCORES * 1024)
    vshc = nu_pad // NCORES

    fp8 = ml_dtypes.float8_e4m3
    Eu = np.zeros((nu_pad, D), dtype=np.float32)
    Eu[:nu] = E[uniq] * np.float32(16.0)
    # (nu_pad, D) -> (NCORES, 128, 2, vshc): [c, p, ch, r] = Eu[c*vshc+r, ch*128+p]
    et = np.ascontiguousarray(
        Eu.reshape(NCORES, vshc, 2, 128).transpose(0, 3, 2, 1)).astype(fp8)
    gm16 = np.zeros((D, 16), dtype=np.float32)
    gm16[:, :3] = gmat * np.float32(16.0)
    gm = np.ascontiguousarray(
        gm16.reshape(2, 128, 16).transpose(1, 0, 2)).astype(fp8)

    p1 = _build_p1(vshc)
    in1 = [{"etab": et[c], "gmat": gm} for c in range(NCORES)]
    res1 = run_bass_kernel_spmd(p1, in1, core_ids=list(range(NCORES)), **run_kw1)
    proj = np.concatenate([res1.results[c]["projout"] for c in range(NCORES)],
                          axis=1) / np.float32(256.0)         # (3, nu_pad)

    # ---- host gather (pure indexing glue) ----
    inv2 = inv.reshape(T + 1, K)
    b_g = proj[0][inv2]      # (1025, 64)
    u_g = proj[1][inv2]
    v_g = proj[2][inv2]

    p2 = _build_p2()
    mlogs = -np.log(s)
    ident = np.eye(128, dtype=np.float32)
    eye64s = (np.eye(K, dtype=np.float32) / np.float32(s))
    obsTf = obs.reshape(NCORES, NT, 2, 128).transpose(0, 3, 2, 1)  # c,p,ch,t
    gef = np.ascontiguousarray(g_e0.reshape(2, 128).T.reshape(128, 2))
    qi = np.tile(np.eye(K, dtype=np.float32), (2, NPAIR))     # (128, NPAIR*K)
    iones = np.concatenate([np.eye(K, dtype=np.float32),
                            np.ones((1, K), np.float32)], axis=0)  # (65, 64)
    tt = (4 * (np.arange(NBLK) % NPAIR) + np.arange(NBLK) // NPAIR)  # t_top(beta)

    in2 = []
    for c in range(NCORES):
        ta = c * NT
        u_loc = u_g[ta : ta + NT]          # (128, 64)
        v_loc = v_g[ta + 1 : ta + NT + 1]  # (128, 64)
        blob = np.zeros((128, B_END), dtype=np.uint8)

        def put(off, arr):
            a8 = np.ascontiguousarray(arr).view(np.uint8).reshape(128, -1)
            blob[:, off : off + a8.shape[1]] = a8

        cols = np.empty((128, 4), dtype=np.float32)
        cols[:, 0] = np.float32(ct / 2)
        cols[:, 1] = np.float32(ce / 2)
        cols[:, 2] = np.float32(2.0 + 2.0 * mlogs)
        cols[:, 3] = 1.0
        addt = np.zeros((128, K), dtype=np.float32)
        if c == NCORES - 1:
            cols[K:, 3] = 0.0
            addt[K:] = eye64s
        put(B_ID, ident)
        put(B_BT, np.ascontiguousarray(b_g[ta : ta + NT].astype(np.float32)))
        put(B_CV, cols)
        put(B_ADD, addt)
        put(B_OBS, np.ascontiguousarray(obsTf[c]).astype(bf16))
        put(B_GE, gef.astype(bf16))


        uvt = np.empty((65, 4096), dtype=np.float32)
        uvb = np.empty((65, 4096), dtype=np.float32)
        uvt[:K] = np.broadcast_to(
            u_loc[tt].T[:, :, None], (K, NBLK, K)).reshape(K, NBLK * K)
        uvt[K] = v_loc[tt].reshape(-1)
        uvb[:K] = np.broadcast_to(
            u_loc[tt + K].T[:, :, None], (K, NBLK, K)).reshape(K, NBLK * K)
        uvb[K] = v_loc[tt + K].reshape(-1)
        # interleave into 2048-col chunks: [top_it (1024) | bot_it (1024)]
        uvarr = np.empty((65, 8256), dtype=np.float32)
        for ck in range(4):
            uvarr[:, ck * 2048 : ck * 2048 + 1024] = uvt[:, ck * 1024 : (ck + 1) * 1024]
            uvarr[:, ck * 2048 + 1024 : (ck + 1) * 2048] = uvb[:, ck * 1024 : (ck + 1) * 1024]
        uvarr[:, 8192:8256] = iones
        uvarr = uvarr.astype(bf16)
        in2.append({"blobin": blob, "uvin": np.ascontiguousarray(uvarr),
                    "qinit": np.ascontiguousarray(qi).astype(bf16)})
    res2 = run_bass_kernel_spmd(p2, in2, core_ids=list(range(NCORES)), **run_kw2)

    # ---- host combine in f64 ----
    P = np.eye(K, dtype=np.float64)
    acc = 0.0
    for c in range(NCORES):
        qo = res2.results[c]["qout"].astype(np.float64)
        for sc in range(NSUB):
            g, h = sc % NPAIR, sc // NPAIR
            Q = qo[h * K : (h + 1) * K, g * K : (g + 1) * K]
            P = P @ Q.T
            m = np.abs(P).max()
            P /= m
            acc += np.log(m)
    emit2_last = res2.results[NCORES - 1]["emitout"][NT - 1].astype(np.float64)
    emit_last = (emit2_last + 1.0) / 2.0
    z = P.sum(axis=0) @ np.exp(emit_last)
    ans = np.log(z) + acc + NSUB * LSUB * NCORES * np.log(np.float64(s))
    return np.array([ans], dtype=np.float32), res1, res2


def kernel(**inputs):
    ans, _, _ = _run_launches(inputs)
    return ans


def profiled_run(inputs):
    """Run both launches with NTFF tracing; return summed exec ns (or None)."""
    import sys as _sys
    import types as _types
    try:
        if "antenv.axon_hooks" not in _sys.modules:
            from trn_agent_boot.trn_boot import _ntff_profile_via_ctypes
            hook = _ntff_profile_via_ctypes("/opt/axon/libaxon_pjrt.so")
            mod = _types.ModuleType("antenv.axon_hooks")
            mod.get_axon_ntff_profile_hook = lambda: hook
            mod.set_axon_ntff_profile_hook = lambda h: None
            _sys.modules["antenv.axon_hooks"] = mod
            import antenv
            antenv.axon_hooks = mod
    except Exception as e:
        print(f"profile shim unavailable: {e}")
        return None
    kw = {"trace": True, "trace_cores": [0]}
    ans, res1, res2 = _run_launches(inputs, run_kw1=dict(kw), run_kw2=dict(kw))
    print("profiled answer:", ans)
    for name, r in (("P1", res1), ("P2", res2)):
        tr = r.instructions_and_trace
        print(f"{name}: exec_time_ns={r.exec_time_ns}"
              + (f" trace={tr[1]}" if tr else ""))
    if res1.exec_time_ns is None or res2.exec_time_ns is None:
        return None
    return res1.exec_time_ns + res2.exec_time_ns
